# revision 3
# baseline (speedup 1.0000x reference)
"""Trainium2 Bass kernel: loss = 0.001 * ||diag(d^T d) - I||_F.

Contract: kernel(**inputs) takes the FULL input d [262144, 256] f32 and
returns the FULL scalar output, matching reference():

    col_sq = sum(d * d, axis=0)            # [256]
    loss   = 0.001 * sqrt(sum((col_sq - 1)^2))

Strategy (8 NeuronCores, row-sharded data parallel):
  - Shard d row-wise into 8 shards of [32768, 256], one per core.
  - Per core, stream [128, g*256] tiles from HBM and accumulate the
    per-column sum of squares.  Two compute paths:
      * "pe"  (default): gram-diagonal on the TensorEngine.  For each
        [128, 256] sub-tile S, matmul(S[:, 0:128].T @ S) and
        matmul(S[:, 128:256].T @ S) accumulate into two PSUM tiles whose
        diagonals are exactly the per-column sums of squares.  Squaring
        and the partition-dim reduction happen inside the PE MACs; the
        only non-PE work is the final PSUM->SBUF evacuation.  Uses
        float32r (full-rate fp32 path, 1 cycle/row for moving dim >=256).
      * "act": ScalarEngine Square + VectorEngine binary-tree folds into
        a [128, 256] accumulator, then a ones-vector fp32 matmul for the
        partition-dim reduction.  Exact fp32, used as numerics fallback.
  - Host: sum the 8 per-core partials in float64, extract diagonals
    (pe path), and finish the tiny scalar reduction.

DMA schedule (the kernel is HBM-bound; burst roofline = 32 MiB /
~360 GB/s = 93.2 us/core; sustained streaming throttles toward
~330 GB/s): each 4 MiB tile is fetched as two 2 MiB halves issued
concurrently on the two HWDGE rings (SP via nc.sync, ACT via nc.scalar)
so per-DMA fixed/completion costs overlap; the tile plan tapers to
[16, 8, 4, 2, 2] row-groups at the end so almost no PE work remains
after the last byte lands; both PSUM evacuations (DVE + ACT) land in
one SBUF tile written out by a single 256 KiB DMA.  Interleaved HW
A/B medians: 92-102 us/pass depending on thermal regime, equal to the
g=16 schedule within noise, with half the DMA instructions (25 vs 48).
Timeline cost model: 100.6 us single-shot, of which 93.9 us DMA-busy.
Rel err vs the float64 reference: ~1e-7 — float32r's reduced-mantissa
products average out over the 262144-row reduction.
"""

import os
import sys

import numpy as np

for _p in ("/opt/trn_rl_repo",):
    if _p not in sys.path and os.path.isdir(_p):
        sys.path.insert(0, _p)

N_ROWS = 262144
M = 256
N_CORES = 8
SHARD = N_ROWS // N_CORES  # 32768 rows per core
P = 128  # SBUF partitions
G = 16  # [128, 256] sub-tiles per DMA'd big tile (2 MiB per DMA)

# Stash of the most recent BassKernelResults (test.py reads exec_time_ns).
LAST_RESULT = None

# Default build configuration for the graded kernel() entry point.
# g=32: 4 MiB tiles, split across both HWDGE rings (SP + ACT) as two 2 MiB
# half-DMAs issued concurrently; taper=1 shrinks the last tiles so almost
# no PE work remains after the final DMA byte lands.  g=16 and g=32 are
# within measurement noise on HW (both sit at the ~360 GB/s burst /
# ~330 GB/s sustained per-core DMA wall; interleaved A/B medians 93-102 us
# depending on thermal regime), but g=32 halves the DMA instruction count
# (24 stream DMAs + 1 output DMA vs 46 + 2), which minimizes per-DMA
# fixed/notification overheads.  12-DMA schedules (g=64, or g=32 unsplit)
# measure ~5 us SLOWER sustained — 24 DMAs is the sweet spot.  outsplit=0
# merges both PSUM evacuations into one SBUF tile and a single 256 KiB
# output DMA: timeline model favors it by ~0.3 us over split outputs and
# it halves the output-DMA count.
CONFIG = dict(g=32, bufs=4, nqueues=2, split=1, taper=1, outsplit=0)

_programs = {}


def _build(
    path, bench_reps=1, g=G, bufs=4, nqueues=1, split=0, taper=0, outsplit=0
):
    import concourse.bacc as bacc
    import concourse.tile as tile
    from concourse import mybir

    f32 = mybir.dt.float32
    # float32r = fp32 storage on the TensorEngine's full-rate path (reduced
    # internal mantissa).  numpy-side dtype is float32 either way.
    d_dt = mybir.dt.float32r if path == "pe" else f32
    # Bacc (not raw Bass): its compile() legalizes multi-wait instructions
    # into event semaphores — TRN2 instructions carry at most one sem wait.
    nc = bacc.Bacc(trn_type="TRN2")
    d = nc.dram_tensor("d", [SHARD, M], d_dt, kind="ExternalInput").ap()
    # Tile plan: list of per-DMA sub-tile counts.  Uniform by default; with
    # taper=1 the tail shrinks so the PE work left after the final DMA lands
    # (the single-pass tail) is small.
    n_units = SHARD // P  # 256 [128, 256]-row-groups per shard
    if taper:
        # Geometric tail [g/2, g/4, ..., 4, 2, 2] sums to exactly g, so the
        # body stays g-aligned and the last DMA is only 2 row-groups
        # (256 KiB): the post-last-DMA PE burst shrinks ~g/2-fold.
        tail = []
        t = g // 2
        while t >= 4:
            tail.append(t)
            t //= 2
        tail += [2, 2]
        assert sum(tail) == g
        glist = [g] * (n_units // g - 1) + tail
    else:
        assert n_units % g == 0
        glist = [g] * (n_units // g)
    assert sum(glist) == n_units
    # [p, u, m]: partition p, row-group u, column m.  Row-inner mapping (the
    # u groups of a tile are consecutive rows per partition) makes each
    # partition's DMA read g*1KiB contiguous.  Any row->partition assignment
    # is valid: the gram diagonals sum over all rows regardless.
    dv = d.rearrange("(p u) m -> p u m", p=P, u=n_units)

    if path == "pe":
        out = nc.dram_tensor("out", [P, 2 * M], f32, kind="ExternalOutput").ap()
        with tile.TileContext(nc) as tc:
            with (
                tc.tile_pool(name="xs", bufs=bufs) as xs,
                tc.tile_pool(name="ps", bufs=1, space="PSUM") as ps,
                tc.tile_pool(name="outs", bufs=1) as outs,
            ):
                ps_a = ps.tile([P, M], f32)
                ps_b = ps.tile([P, M], f32)
                # Two HWDGE rings: SP (nc.sync) and ACT (nc.scalar).
                # Alternating big-tile DMAs between them overlaps the
                # per-DMA completion latency that serializes a single ring.
                dma_engines = [nc.sync, nc.scalar][:nqueues]

                def full_pass():
                    u0 = 0
                    for t, gt in enumerate(glist):
                        xt = xs.tile([P, gt, M], mybir.dt.float32r)
                        src = dv[:, u0 : u0 + gt, :]
                        if split == 3 and gt >= 3:
                            # Three generation paths: SP + ACT (HWDGE rings)
                            # and GpSimd (SWDGE) — each SDMA engine
                            # round-robins between their internal queues.
                            a = (gt + 2) // 3
                            b = 2 * a if 2 * a <= gt else gt
                            nc.sync.dma_start(
                                out=xt[:, 0:a, :], in_=src[:, 0:a, :]
                            )
                            nc.scalar.dma_start(
                                out=xt[:, a:b, :], in_=src[:, a:b, :]
                            )
                            if b < gt:
                                nc.gpsimd.dma_start(
                                    out=xt[:, b:gt, :], in_=src[:, b:gt, :]
                                )
                        elif split and gt >= 2:
                            h = gt // 2
                            nc.sync.dma_start(
                                out=xt[:, 0:h, :], in_=src[:, 0:h, :]
                            )
                            nc.scalar.dma_start(
                                out=xt[:, h:gt, :], in_=src[:, h:gt, :]
                            )
                        else:
                            eng = dma_engines[t % len(dma_engines)]
                            eng.dma_start(out=xt, in_=src)
                        for gg in range(gt):
                            sub = xt[:, gg, :]
                            first = t == 0 and gg == 0
                            last = t == len(glist) - 1 and gg == gt - 1
                            nc.tensor.matmul(
                                ps_a, sub[:, 0:P], sub, start=first, stop=last
                            )
                            nc.tensor.matmul(
                                ps_b, sub[:, P:M], sub, start=first, stop=last
                            )
                        u0 += gt

                if bench_reps > 1:
                    # Benchmark mode: repeat the whole streaming pass in a HW
                    # loop; start=True re-clears PSUM so results stay valid.
                    # PE body is >256 instructions (one IRAM block), so hint
                    # the back-edge to avoid a ~4us I-fetch stall per pass.
                    with tc.For_i(
                        0, bench_reps, 1, hint_engines=(mybir.EngineType.PE,)
                    ):
                        full_pass()
                else:
                    full_pass()
                if outsplit:
                    # Two SEPARATE SBUF tiles so Tile's tile-granular
                    # dependency tracking lets each evac->DMA chain fire
                    # independently: DVE evac ps_a -> SP-ring DMA runs in
                    # parallel with ACT evac ps_b -> ACT-ring DMA.
                    o_a = outs.tile([P, M], f32)
                    o_b = outs.tile([P, M], f32)
                    nc.vector.tensor_copy(o_a, ps_a)
                    nc.sync.dma_start(out=out[:, 0:M], in_=o_a)
                    nc.scalar.activation(
                        o_b, ps_b, mybir.ActivationFunctionType.Copy
                    )
                    nc.scalar.dma_start(out=out[:, M : 2 * M], in_=o_b)
                else:
                    o = outs.tile([P, 2 * M], f32)
                    # Evacuate the two PSUM tiles on different engines so
                    # the copies overlap in the single-pass tail.
                    nc.vector.tensor_copy(o[:, 0:M], ps_a)
                    nc.scalar.activation(
                        o[:, M : 2 * M], ps_b,
                        mybir.ActivationFunctionType.Copy,
                    )
                    nc.sync.dma_start(out=out, in_=o)

        def post(outs_np):
            s = np.sum(np.asarray(outs_np, dtype=np.float64), axis=0)  # [128, 512]
            a, b = s[:, :M], s[:, M:]
            idx = np.arange(P)
            colsq = np.concatenate([a[idx, idx], b[idx, P + idx]])
            return colsq

    elif path == "act":
        out = nc.dram_tensor("out", [1, M], f32, kind="ExternalOutput").ap()
        with tile.TileContext(nc) as tc:
            with (
                tc.tile_pool(name="xs", bufs=3) as xs,
                tc.tile_pool(name="sq", bufs=2) as sqp,
                tc.tile_pool(name="acc", bufs=1) as accp,
                tc.tile_pool(name="ps", bufs=1, space="PSUM") as ps,
                tc.tile_pool(name="outs", bufs=1) as outs,
            ):
                acc = accp.tile([P, M], f32)
                ones = accp.tile([P, 1], f32)
                nc.vector.memset(acc, 0.0)
                nc.vector.memset(ones, 1.0)

                def full_pass():
                    u0 = 0
                    for gt in glist:
                        xt = xs.tile([P, gt * M], f32)
                        nc.sync.dma_start(
                            out=xt.rearrange("p (g m) -> p g m", g=gt),
                            in_=dv[:, u0 : u0 + gt, :],
                        )
                        sq = sqp.tile([P, gt * M], f32)
                        nc.scalar.activation(
                            sq, xt, mybir.ActivationFunctionType.Square
                        )
                        h = gt * M // 2
                        while h >= M:
                            nc.vector.tensor_add(
                                sq[:, :h], sq[:, :h], sq[:, h : 2 * h]
                            )
                            h //= 2
                        nc.vector.tensor_add(acc, acc, sq[:, :M])
                        u0 += gt

                if bench_reps > 1:
                    with tc.For_i(0, bench_reps, 1):
                        full_pass()
                else:
                    full_pass()
                # Partition-dim reduction: [1, 256] = ones[128,1].T @ acc.
                psum1 = ps.tile([1, M], f32)
                nc.tensor.matmul(psum1, ones, acc, start=True, stop=True)
                o = outs.tile([1, M], f32)
                nc.vector.tensor_copy(o, psum1)
                nc.sync.dma_start(out=out, in_=o)

        def post(outs_np):
            s = np.sum(np.asarray(outs_np, dtype=np.float64), axis=0)  # [1, 256]
            return s[0]

    else:
        raise ValueError(f"unknown path {path!r}")

    nc.compile()
    return nc, post


def _get_program(path):
    if path not in _programs:
        _programs[path] = _build(path, **CONFIG)
    return _programs[path]


def kernel(d):
    global LAST_RESULT
    from concourse.bass_utils import run_bass_kernel_spmd

    d_np = np.ascontiguousarray(np.asarray(d, dtype=np.float32))
    assert d_np.shape == (N_ROWS, M), d_np.shape

    path = os.environ.get("BASS_KERNEL_PATH", "pe")
    nc, post = _get_program(path)

    shards = d_np.reshape(N_CORES, SHARD, M)
    in_maps = [{"d": np.ascontiguousarray(shards[i])} for i in range(N_CORES)]
    try:
        res = run_bass_kernel_spmd(nc, in_maps, core_ids=list(range(N_CORES)))
    except ModuleNotFoundError:
        # BASS_TRACE=1 under axon needs antenv.axon_hooks, which slim
        # containers lack — rerun untraced rather than crash.
        os.environ["BASS_NEVER_TRACE"] = "1"
        res = run_bass_kernel_spmd(nc, in_maps, core_ids=list(range(N_CORES)))
    LAST_RESULT = res

    colsq = post([r["out"] for r in res.results])
    loss = 0.001 * np.sqrt(np.sum((colsq - 1.0) ** 2))
    return np.asarray(loss, dtype=np.float32)

